# revision 23
# baseline (speedup 1.0000x reference)
"""Data-parallel attention kernel for Trainium2 (8 NeuronCores).

Reference computation (per batch item b):
    scores[q, k] = sum_{hw} query[b, hw, q] * keys[b, hw, k]     (C=256, HW=4096)
    attn = softmax_k(scores)
    out[b, q, hw] = sum_k attn[q, k] * values[b, hw, k]

Sharding: batch axis (B=32) split across 8 cores, 4 items per core, no
cross-core communication.

v2 design (vs the 182us f32-input baseline): inputs are cast to f16 on
the HOST inside kernel(), so the device streams 2-byte elements.  Per-core
HBM traffic drops 58.7MB -> 33.6MB (Q/K/V reads 25.2MB + f16 output
8.4MB), moving the bottleneck from DMA (~164us floor) to the PE
(~119us of matmul+transpose work).  f16 matmuls run at full PE rate
(1 elem/cell/cycle, same as bf16) and 11-bit mantissas give BETTER
accuracy than the old f32r/bf16 mix (measured rel err 1.2e-3 vs 1.8e-3).

Per-core per-item plan:
  S phase:  f16 matmuls, contraction over hw streamed in 8 groups of 512
            rows, accumulating into one PSUM bank per q-block.
  softmax:  DVE row-max (negated) -> ACT exp(in + bias) with accumulated
            row sums -> DVE reciprocal. Normalization is folded into the
            O-phase epilogue, so A stays unnormalized f16.
  O phase:  V streamed in 8 groups of 512 rows, PE-transposed
            ([hw,k] -> [k,hw]) via identity matmuls, then f16 matmuls
            A @ V^T accumulated over the 2 k-chunks; the epilogue (split
            ACT/DVE) scales rows by 1/rowsum during the PSUM->SBUF copy
            and writes f16 output (upcast to f32 on the host).

Scheduling notes:
  - Input DMAs ride the single gpsimd SWDGE queue in consumption order
    (QK front-loaded 1.5:1 vs V); with f16 the DMA stream (~70us of
    input) runs well ahead of the PE, so deep tile pools (SBUF holds ~3
    batches of Q/K) decouple the two.
  - Output DMAs ride the HWDGE ring (nc.sync) so data-dependent waits
    never block input prefetch.
"""

import numpy as np

import concourse.bass as bass
import concourse.tile as tile
from concourse import bacc, mybir
from concourse.bass_utils import run_bass_kernel_spmd
from contextlib import ExitStack

B, H, W, C = 32, 64, 64, 256
N_CORES = 8
B_LOC = B // N_CORES          # 4 batch items per core
HW = H * W                    # 4096
P = 128                       # partitions
N_CHUNK = HW // P             # 32 chunks of 128 hw-rows
SG = 4                        # chunks per S-phase group (512 hw rows)
VG = 4                        # chunks per O-phase group (512 hw rows)
N_SGRP = N_CHUNK // SG        # 8
N_VGRP = N_CHUNK // VG        # 8
QB = C // P                   # 2 q-blocks
KC = C // P                   # 2 k-chunks

F32 = mybir.dt.float32
F16 = mybir.dt.float16

_CACHE = {}


def _build():
    nc = bacc.Bacc("TRN2", target_bir_lowering=False, debug=False,
                   num_devices=N_CORES)
    q_ext = nc.dram_tensor("query", [B_LOC, H, W, C], F16,
                           kind="ExternalInput").ap()
    k_ext = nc.dram_tensor("keys", [B_LOC, H, W, C], F16,
                           kind="ExternalInput").ap()
    v_ext = nc.dram_tensor("values", [B_LOC, H, W, C], F16,
                           kind="ExternalInput").ap()
    # Output written in [g, c, p] block order (hw = p*32 + g*VG + c);
    # the host unscrambles. 1KB-contiguous pieces per partition line.
    o_ext = nc.dram_tensor("out", [B_LOC, C, N_VGRP, VG, P], F16,
                           kind="ExternalOutput").ap()

    # [b, hw, c] -> [b, p, n, c] with hw = p*32 + n: each partition line
    # covers consecutive DRAM rows, so a group DMA moves SG*512B = 2KB
    # contiguous pieces (4x fewer SWDGE descriptors than the (n p) split,
    # whose pieces are single 512B c-rows).  The S/O contractions sum
    # over all hw, so the chunk->partition assignment is free.
    qv = q_ext.rearrange("b h w c -> b (h w) c").rearrange(
        "b (p n) c -> b p n c", p=P)
    kv = k_ext.rearrange("b h w c -> b (h w) c").rearrange(
        "b (p n) c -> b p n c", p=P)
    vv = v_ext.rearrange("b h w c -> b (h w) c").rearrange(
        "b (p n) c -> b p n c", p=P)

    with tile.TileContext(nc) as tc, ExitStack() as ctx:
        qk_pool = ctx.enter_context(tc.tile_pool(name="qk", bufs=28))
        vb_pool = ctx.enter_context(tc.tile_pool(name="vb", bufs=16))
        vt_pool = ctx.enter_context(tc.tile_pool(name="vt", bufs=8))
        a_pool = ctx.enter_context(tc.tile_pool(name="a", bufs=3))
        at_pool = ctx.enter_context(tc.tile_pool(name="at", bufs=4))
        o_pool = ctx.enter_context(tc.tile_pool(name="o", bufs=6))
        stat_pool = ctx.enter_context(tc.tile_pool(name="stat", bufs=2 * B_LOC))
        singles = ctx.enter_context(tc.tile_pool(name="singles", bufs=1))
        ps_s = ctx.enter_context(tc.tile_pool(name="ps_s", bufs=2, space="PSUM"))
        ps_vt = ctx.enter_context(tc.tile_pool(name="ps_vt", bufs=3, space="PSUM"))
        ps_o = ctx.enter_context(tc.tile_pool(name="ps_o", bufs=3, space="PSUM"))

        # Identity for PE transposes, embedded in the NEFF as a Const
        # DRAM tensor (loaded at model-load time, not exec time).
        ident_dram = nc.inline_tensor(
            np.eye(P, dtype=np.float16), name="ident_const")
        ident = singles.tile([P, P], F16)

        def issue_qk_group(b, g, eng=None):
            eng = eng or nc.gpsimd
            q_t = qk_pool.tile([P, SG, C], F16, tag="q", name=f"q_t_{b}_{g}")
            eng.dma_start(out=q_t[:],
                          in_=qv[b, :, g * SG:(g + 1) * SG, :])
            k_t = qk_pool.tile([P, SG, C], F16, tag="k", name=f"k_t_{b}_{g}")
            eng.dma_start(out=k_t[:],
                          in_=kv[b, :, g * SG:(g + 1) * SG, :])
            return (q_t, k_t)

        def issue_v_group(b, g):
            vb_t = vb_pool.tile([P, VG, C], F16, tag="vb",
                                name=f"vb_t_{b}_{g}")
            nc.gpsimd.dma_start(out=vb_t[:],
                                in_=vv[b, :, g * VG:(g + 1) * VG, :])
            return vb_t

        # Input DMAs ride the single gpsimd SWDGE queue (program order);
        # issue in consumption order with the Q,K stream front-loaded.
        # First two QK group-pairs ride the ACT HWDGE ring: it needs no
        # descriptor-generation spin-up, so the stream starts while the
        # SWDGE Q7 warms up (~6us).  ACT has no queued compute yet, so
        # nothing can stall behind these.
        qk_by_batch = {0: [issue_qk_group(0, g, eng=nc.scalar if g < 2 else None)
                           for g in range(N_SGRP)]}
        nc.sync.dma_start(out=ident[:], in_=ident_dram.ap())

        qk_flat = [(bb, g) for bb in range(1, B_LOC) for g in range(N_SGRP)]
        qi = 0

        for b in range(B_LOC):
            # Interleaved input issue for this phase.
            v_tiles = []
            for g in range(N_VGRP):
                v_tiles.append(issue_v_group(b, g))
                npop = 2 if g % 2 == 0 else 1
                for _ in range(npop):
                    if qi < len(qk_flat):
                        bb, gg = qk_flat[qi]
                        qi += 1
                        qk_by_batch.setdefault(bb, []).append(
                            issue_qk_group(bb, gg))

            # ---- S = Q^T K (f16), accumulate over hw ----
            s_ps = [ps_s.tile([P, C], F32, tag="ps_s", name=f"s_ps_{b}_{qb}")
                    for qb in range(QB)]
            for g in range(N_SGRP):
                q_t, k_t = qk_by_batch[b][g]
                for c in range(SG):
                    for qb in range(QB):
                        nc.tensor.matmul(
                            s_ps[qb][:],
                            lhsT=q_t[:, c, qb * P:(qb + 1) * P],
                            rhs=k_t[:, c, :],
                            start=(g == 0 and c == 0),
                            stop=(g == N_SGRP - 1 and c == SG - 1),
                        )

            # ---- softmax over k (free axis) ----
            negmax = stat_pool.tile([P, QB, 1], F32, tag="negmax")
            rowsum = stat_pool.tile([P, QB, 1], F32, tag="rowsum")
            recip = stat_pool.tile([P, QB, 1], F32, tag="recip")
            a_sb = a_pool.tile([P, QB, C], F16, tag="a")
            for qb in range(QB):
                nc.vector.tensor_reduce(
                    out=negmax[:, qb, :], in_=s_ps[qb][:],
                    axis=mybir.AxisListType.X, op=mybir.AluOpType.max,
                    negate=True)
                nc.scalar.activation(
                    out=a_sb[:, qb, :], in_=s_ps[qb][:],
                    func=mybir.ActivationFunctionType.Exp,
                    bias=negmax[:, qb, :], scale=1.0,
                    accum_out=rowsum[:, qb, :])
                nc.vector.reciprocal(out=recip[:, qb, :], in_=rowsum[:, qb, :])

            # ---- V^T via PE transposes, pipelined one group ahead ----
            def vt_group(g):
                vb_t = v_tiles[g]
                vt_ps = ps_vt.tile([P, KC, VG, P], F16, tag="ps_vt")
                for c in range(VG):
                    for kc in range(KC):
                        nc.tensor.transpose(
                            out=vt_ps[:, kc, c, :],
                            in_=vb_t[:, c, kc * P:(kc + 1) * P],
                            identity=ident[:])
                vt_sb = vt_pool.tile([P, KC, VG, P], F16, tag="vt")
                # Alternate copy engine so this stage never stacks up on
                # one engine in the PE-paced tail.
                if g % 2 == 0:
                    nc.vector.tensor_copy(out=vt_sb[:], in_=vt_ps[:])
                else:
                    nc.scalar.copy(out=vt_sb[:], in_=vt_ps[:])
                return vt_sb

            # Group 0's V-transposes are emitted BEFORE the A^T
            # transposes: A^T waits on the softmax exp, and the in-order
            # Tensor queue would otherwise idle the PE during that wait.
            vt_cur = vt_group(0)

            # ---- A^T via PE transposes: at[:, kc, qb, :] = A[qb-block, kc-chunk]^T
            at_ps = ps_s.tile([P, KC, QB, P], F16, tag="ps_s")
            for kc in range(KC):
                for qb in range(QB):
                    nc.tensor.transpose(
                        out=at_ps[:, kc, qb, :],
                        in_=a_sb[:, qb, kc * P:(kc + 1) * P],
                        identity=ident[:])
            at_sb = at_pool.tile([P, KC, QB, P], F16, tag="at")
            nc.vector.tensor_copy(out=at_sb[:], in_=at_ps[:])

            # ---- O = A @ V^T, f16, streamed over hw groups ----
            for g in range(N_VGRP):
                vt_sb = vt_cur
                # Emit next group's transposes ahead of this group's
                # matmuls so the PE always has transpose work queued
                # while epilogue/copy stages drain.
                if g + 1 < N_VGRP:
                    vt_cur = vt_group(g + 1)
                paired = g < N_VGRP - 2
                if g % 2 == 0:
                    # Pair two groups per output tile so each output DMA
                    # writes 2KB-contiguous pieces per partition line.
                    # The last two groups write singly so the final
                    # drain DMA is half-size.
                    o_sbs = [o_pool.tile([P, 2, VG * P], F16, tag=f"o{qb}",
                                          name=f"o_sb_{b}_{g}_{qb}")
                             for qb in range(QB)]
                for qb in range(QB):
                    o_ps = ps_o.tile([P, VG * P], F32, tag="ps_o")
                    for kc in range(KC):
                        nc.tensor.matmul(
                            o_ps[:],
                            lhsT=at_sb[:, kc, qb, :],
                            rhs=vt_sb[:, kc, :, :].rearrange("p c x -> p (c x)"),
                            start=(kc == 0), stop=(kc == KC - 1),
                        )
                    # Split epilogues between ACT and DVE to balance load.
                    if qb == 0:
                        nc.scalar.activation(
                            out=o_sbs[qb][:, g % 2, :], in_=o_ps[:],
                            func=mybir.ActivationFunctionType.Copy,
                            scale=recip[:, qb, :])
                    else:
                        nc.vector.tensor_scalar_mul(
                            o_sbs[qb][:, g % 2, :], o_ps[:], recip[:, qb, :])
                    if paired and g % 2 == 1:
                        nc.sync.dma_start(
                            out=o_ext[b, qb * P:(qb + 1) * P, g - 1:g + 1, :, :],
                            in_=o_sbs[qb][:].rearrange(
                                "q t (c p) -> q t c p", p=P))
                    elif not paired:
                        nc.sync.dma_start(
                            out=o_ext[b, qb * P:(qb + 1) * P, g, :, :],
                            in_=o_sbs[qb][:, g % 2, :].rearrange(
                                "q (c p) -> q c p", p=P))

    nc.compile()
    return nc


def _get_nc():
    if "nc" not in _CACHE:
        _CACHE["nc"] = _build()
    return _CACHE["nc"]


def prep_in_maps(query, keys, values):
    """Host-side prep: cast f32 -> f16 and slice the batch across cores."""
    q16 = np.ascontiguousarray(np.asarray(query)).astype(np.float16)
    k16 = np.ascontiguousarray(np.asarray(keys)).astype(np.float16)
    v16 = np.ascontiguousarray(np.asarray(values)).astype(np.float16)
    in_maps = []
    for i in range(N_CORES):
        sl = slice(i * B_LOC, (i + 1) * B_LOC)
        in_maps.append({
            "query": np.ascontiguousarray(q16[sl]),
            "keys": np.ascontiguousarray(k16[sl]),
            "values": np.ascontiguousarray(v16[sl]),
        })
    return in_maps


def assemble_out(res):
    """Host-side postprocess: gather per-core f16 outputs, unscramble the
    hw axis (written as [g, c, p] blocks; hw = p*32 + g*VG + c), -> f32."""
    parts = []
    for i in range(N_CORES):
        arr = res.results[i]["out"]          # [B_LOC, C, N_VGRP, VG, P]
        arr = arr.transpose(0, 1, 4, 2, 3).reshape(B_LOC, C, H, W)
        parts.append(arr.astype(np.float32))
    return np.concatenate(parts, axis=0)


def kernel(query, keys, values):
    assert np.asarray(query).shape == (B, H, W, C)
    nc = _get_nc()
    in_maps = prep_in_maps(query, keys, values)
    res = run_bass_kernel_spmd(nc, in_maps, core_ids=list(range(N_CORES)))
    return assemble_out(res)


# revision 24
# speedup vs baseline: 1.1336x; 1.1336x over previous
"""Data-parallel attention kernel for Trainium2 (8 NeuronCores).

Reference computation (per batch item b):
    scores[q, k] = sum_{hw} query[b, hw, q] * keys[b, hw, k]     (C=256, HW=4096)
    attn = softmax_k(scores)
    out[b, q, hw] = sum_k attn[q, k] * values[b, hw, k]

Sharding: batch axis (B=32) split across 8 cores, 4 items per core, no
cross-core communication.

v2 design (vs the 182us f32-input baseline): inputs are cast to f16 on
the HOST inside kernel(), so the device streams 2-byte elements.  Per-core
HBM traffic drops 58.7MB -> 33.6MB (Q/K/V reads 25.2MB + f16 output
8.4MB), moving the bottleneck from DMA (~164us floor) to the PE
(~119us of matmul+transpose work).  f16 matmuls run at full PE rate
(1 elem/cell/cycle, same as bf16) and 11-bit mantissas give BETTER
accuracy than the old f32r/bf16 mix (measured rel err 1.2e-3 vs 1.8e-3).

Per-core per-item plan:
  S phase:  f16 matmuls, contraction over hw streamed in 8 groups of 512
            rows, accumulating into one PSUM bank per q-block.
  softmax:  DVE row-max (negated) -> ACT exp(in + bias) with accumulated
            row sums -> DVE reciprocal. Normalization is folded into the
            O-phase epilogue, so A stays unnormalized f16.
  O phase:  V streamed in 8 groups of 512 rows, PE-transposed
            ([hw,k] -> [k,hw]) via identity matmuls, then f16 matmuls
            A @ V^T accumulated over the 2 k-chunks; the epilogue (split
            ACT/DVE) scales rows by 1/rowsum during the PSUM->SBUF copy
            and writes f16 output (upcast to f32 on the host).

Scheduling notes:
  - Input DMAs ride the single gpsimd SWDGE queue in consumption order
    (QK front-loaded 1.5:1 vs V); with f16 the DMA stream (~70us of
    input) runs well ahead of the PE, so deep tile pools (SBUF holds ~3
    batches of Q/K) decouple the two.
  - Output DMAs ride the HWDGE ring (nc.sync) so data-dependent waits
    never block input prefetch.
"""

import numpy as np

import concourse.bass as bass
import concourse.tile as tile
from concourse import bacc, mybir
from concourse.bass_utils import run_bass_kernel_spmd
from contextlib import ExitStack

B, H, W, C = 32, 64, 64, 256
N_CORES = 8
B_LOC = B // N_CORES          # 4 batch items per core
HW = H * W                    # 4096
P = 128                       # partitions
N_CHUNK = HW // P             # 32 chunks of 128 hw-rows
SG = 4                        # chunks per S-phase group (512 hw rows)
VG = 4                        # chunks per O-phase group (512 hw rows)
N_SGRP = N_CHUNK // SG        # 8
N_VGRP = N_CHUNK // VG        # 8
QB = C // P                   # 2 q-blocks
KC = C // P                   # 2 k-chunks

F32 = mybir.dt.float32
F16 = mybir.dt.float16

_CACHE = {}


def _build():
    nc = bacc.Bacc("TRN2", target_bir_lowering=False, debug=False,
                   num_devices=N_CORES)
    q_ext = nc.dram_tensor("query", [B_LOC, H, W, C], F16,
                           kind="ExternalInput").ap()
    k_ext = nc.dram_tensor("keys", [B_LOC, H, W, C], F16,
                           kind="ExternalInput").ap()
    v_ext = nc.dram_tensor("values", [B_LOC, H, W, C], F16,
                           kind="ExternalInput").ap()
    # Output written in [g, c, p] block order (hw = p*32 + g*VG + c);
    # the host unscrambles. 1KB-contiguous pieces per partition line.
    o_ext = nc.dram_tensor("out", [B_LOC, C, N_VGRP, VG, P], F16,
                           kind="ExternalOutput").ap()

    # [b, hw, c] -> [b, p, n, c] with hw = p*32 + n: each partition line
    # covers consecutive DRAM rows, so a group DMA moves SG*512B = 2KB
    # contiguous pieces (4x fewer SWDGE descriptors than the (n p) split,
    # whose pieces are single 512B c-rows).  The S/O contractions sum
    # over all hw, so the chunk->partition assignment is free.
    qv = q_ext.rearrange("b h w c -> b (h w) c").rearrange(
        "b (p n) c -> b p n c", p=P)
    kv = k_ext.rearrange("b h w c -> b (h w) c").rearrange(
        "b (p n) c -> b p n c", p=P)
    vv = v_ext.rearrange("b h w c -> b (h w) c").rearrange(
        "b (p n) c -> b p n c", p=P)

    with tile.TileContext(nc) as tc, ExitStack() as ctx:
        qk_pool = ctx.enter_context(tc.tile_pool(name="qk", bufs=24))
        vb_pool = ctx.enter_context(tc.tile_pool(name="vb", bufs=12))
        vt_pool = ctx.enter_context(tc.tile_pool(name="vt", bufs=8))
        a_pool = ctx.enter_context(tc.tile_pool(name="a", bufs=3))
        at_pool = ctx.enter_context(tc.tile_pool(name="at", bufs=4))
        o_pool = ctx.enter_context(tc.tile_pool(name="o", bufs=6))
        stat_pool = ctx.enter_context(tc.tile_pool(name="stat", bufs=2 * B_LOC))
        singles = ctx.enter_context(tc.tile_pool(name="singles", bufs=1))
        ps_s = ctx.enter_context(tc.tile_pool(name="ps_s", bufs=2, space="PSUM"))
        ps_vt = ctx.enter_context(tc.tile_pool(name="ps_vt", bufs=3, space="PSUM"))
        ps_o = ctx.enter_context(tc.tile_pool(name="ps_o", bufs=3, space="PSUM"))

        # Identity for PE transposes, embedded in the NEFF as a Const
        # DRAM tensor (loaded at model-load time, not exec time).
        ident_dram = nc.inline_tensor(
            np.eye(P, dtype=np.float16), name="ident_const")
        ident = singles.tile([P, P], F16)

        def issue_qk_group(b, g, eng=None):
            eng = eng or nc.gpsimd
            q_t = qk_pool.tile([P, SG, C], F16, tag="q", name=f"q_t_{b}_{g}")
            eng.dma_start(out=q_t[:],
                          in_=qv[b, :, g * SG:(g + 1) * SG, :])
            k_t = qk_pool.tile([P, SG, C], F16, tag="k", name=f"k_t_{b}_{g}")
            eng.dma_start(out=k_t[:],
                          in_=kv[b, :, g * SG:(g + 1) * SG, :])
            return (q_t, k_t)

        def issue_v_group(b, g):
            vb_t = vb_pool.tile([P, VG, C], F16, tag="vb",
                                name=f"vb_t_{b}_{g}")
            nc.gpsimd.dma_start(out=vb_t[:],
                                in_=vv[b, :, g * VG:(g + 1) * VG, :])
            return vb_t

        # Input DMAs ride the single gpsimd SWDGE queue (program order);
        # issue in consumption order with the Q,K stream front-loaded.
        # First two QK group-pairs ride the ACT HWDGE ring: it needs no
        # descriptor-generation spin-up, so the stream starts while the
        # SWDGE Q7 warms up (~6us).  ACT has no queued compute yet, so
        # nothing can stall behind these.
        qk_by_batch = {0: [issue_qk_group(0, g, eng=nc.scalar if g < 2 else None)
                           for g in range(N_SGRP)]}
        nc.sync.dma_start(out=ident[:], in_=ident_dram.ap())

        qk_flat = [(bb, g) for bb in range(1, B_LOC) for g in range(N_SGRP)]
        qi = 0

        for b in range(B_LOC):
            # Interleaved input issue for this phase.
            v_tiles = []
            for g in range(N_VGRP):
                v_tiles.append(issue_v_group(b, g))
                npop = 2 if g % 2 == 0 else 1
                for _ in range(npop):
                    if qi < len(qk_flat):
                        bb, gg = qk_flat[qi]
                        qi += 1
                        qk_by_batch.setdefault(bb, []).append(
                            issue_qk_group(bb, gg))

            # ---- S = Q^T K (f16), accumulate over hw ----
            s_ps = [ps_s.tile([P, C], F32, tag="ps_s", name=f"s_ps_{b}_{qb}")
                    for qb in range(QB)]
            for g in range(N_SGRP):
                q_t, k_t = qk_by_batch[b][g]
                for c in range(SG):
                    for qb in range(QB):
                        nc.tensor.matmul(
                            s_ps[qb][:],
                            lhsT=q_t[:, c, qb * P:(qb + 1) * P],
                            rhs=k_t[:, c, :],
                            start=(g == 0 and c == 0),
                            stop=(g == N_SGRP - 1 and c == SG - 1),
                        )

            # ---- softmax over k (free axis) ----
            negmax = stat_pool.tile([P, QB, 1], F32, tag="negmax")
            rowsum = stat_pool.tile([P, QB, 1], F32, tag="rowsum")
            recip = stat_pool.tile([P, QB, 1], F32, tag="recip")
            a_sb = a_pool.tile([P, QB, C], F16, tag="a")
            for qb in range(QB):
                nc.vector.tensor_reduce(
                    out=negmax[:, qb, :], in_=s_ps[qb][:],
                    axis=mybir.AxisListType.X, op=mybir.AluOpType.max,
                    negate=True)
                nc.scalar.activation(
                    out=a_sb[:, qb, :], in_=s_ps[qb][:],
                    func=mybir.ActivationFunctionType.Exp,
                    bias=negmax[:, qb, :], scale=1.0,
                    accum_out=rowsum[:, qb, :])
                nc.vector.reciprocal(out=recip[:, qb, :], in_=rowsum[:, qb, :])

            # ---- V^T via PE transposes, pipelined one group ahead ----
            def vt_group(g):
                vb_t = v_tiles[g]
                vt_ps = ps_vt.tile([P, KC, VG, P], F16, tag="ps_vt")
                for c in range(VG):
                    for kc in range(KC):
                        nc.tensor.transpose(
                            out=vt_ps[:, kc, c, :],
                            in_=vb_t[:, c, kc * P:(kc + 1) * P],
                            identity=ident[:])
                vt_sb = vt_pool.tile([P, KC, VG, P], F16, tag="vt")
                # Alternate copy engine so this stage never stacks up on
                # one engine in the PE-paced tail.
                if g % 2 == 0:
                    nc.vector.tensor_copy(out=vt_sb[:], in_=vt_ps[:])
                else:
                    nc.scalar.copy(out=vt_sb[:], in_=vt_ps[:])
                return vt_sb

            # Group 0's V-transposes are emitted BEFORE the A^T
            # transposes: A^T waits on the softmax exp, and the in-order
            # Tensor queue would otherwise idle the PE during that wait.
            vt_cur = vt_group(0)

            # ---- A^T via PE transposes: at[:, kc, qb, :] = A[qb-block, kc-chunk]^T
            at_ps = ps_s.tile([P, KC, QB, P], F16, tag="ps_s")
            for kc in range(KC):
                for qb in range(QB):
                    nc.tensor.transpose(
                        out=at_ps[:, kc, qb, :],
                        in_=a_sb[:, qb, kc * P:(kc + 1) * P],
                        identity=ident[:])
            at_sb = at_pool.tile([P, KC, QB, P], F16, tag="at")
            nc.vector.tensor_copy(out=at_sb[:], in_=at_ps[:])

            # ---- O = A @ V^T, f16, streamed over hw groups ----
            for g in range(N_VGRP):
                vt_sb = vt_cur
                # Emit next group's transposes ahead of this group's
                # matmuls so the PE always has transpose work queued
                # while epilogue/copy stages drain.
                if g + 1 < N_VGRP:
                    vt_cur = vt_group(g + 1)
                paired = g < N_VGRP - 2
                if g % 2 == 0:
                    # Pair two groups per output tile so each output DMA
                    # writes 2KB-contiguous pieces per partition line.
                    # The last two groups write singly so the final
                    # drain DMA is half-size.
                    o_sbs = [o_pool.tile([P, 2, VG * P], F16, tag=f"o{qb}",
                                          name=f"o_sb_{b}_{g}_{qb}")
                             for qb in range(QB)]
                for qb in range(QB):
                    o_ps = ps_o.tile([P, VG * P], F32, tag="ps_o")
                    for kc in range(KC):
                        nc.tensor.matmul(
                            o_ps[:],
                            lhsT=at_sb[:, kc, qb, :],
                            rhs=vt_sb[:, kc, :, :].rearrange("p c x -> p (c x)"),
                            start=(kc == 0), stop=(kc == KC - 1),
                        )
                    # Split epilogues between ACT and DVE to balance load.
                    if qb == 0:
                        nc.scalar.activation(
                            out=o_sbs[qb][:, g % 2, :], in_=o_ps[:],
                            func=mybir.ActivationFunctionType.Copy,
                            scale=recip[:, qb, :])
                    else:
                        nc.vector.tensor_scalar_mul(
                            o_sbs[qb][:, g % 2, :], o_ps[:], recip[:, qb, :])
                    if paired and g % 2 == 1:
                        nc.sync.dma_start(
                            out=o_ext[b, qb * P:(qb + 1) * P, g - 1:g + 1, :, :],
                            in_=o_sbs[qb][:].rearrange(
                                "q t (c p) -> q t c p", p=P))
                    elif not paired:
                        nc.sync.dma_start(
                            out=o_ext[b, qb * P:(qb + 1) * P, g, :, :],
                            in_=o_sbs[qb][:, g % 2, :].rearrange(
                                "q (c p) -> q c p", p=P))

    nc.compile()
    return nc


def _get_nc():
    if "nc" not in _CACHE:
        _CACHE["nc"] = _build()
    return _CACHE["nc"]


def prep_in_maps(query, keys, values):
    """Host-side prep: cast f32 -> f16 and slice the batch across cores."""
    q16 = np.ascontiguousarray(np.asarray(query)).astype(np.float16)
    k16 = np.ascontiguousarray(np.asarray(keys)).astype(np.float16)
    v16 = np.ascontiguousarray(np.asarray(values)).astype(np.float16)
    in_maps = []
    for i in range(N_CORES):
        sl = slice(i * B_LOC, (i + 1) * B_LOC)
        in_maps.append({
            "query": np.ascontiguousarray(q16[sl]),
            "keys": np.ascontiguousarray(k16[sl]),
            "values": np.ascontiguousarray(v16[sl]),
        })
    return in_maps


def assemble_out(res):
    """Host-side postprocess: gather per-core f16 outputs, unscramble the
    hw axis (written as [g, c, p] blocks; hw = p*32 + g*VG + c), -> f32."""
    parts = []
    for i in range(N_CORES):
        arr = res.results[i]["out"]          # [B_LOC, C, N_VGRP, VG, P]
        arr = arr.transpose(0, 1, 4, 2, 3).reshape(B_LOC, C, H, W)
        parts.append(arr.astype(np.float32))
    return np.concatenate(parts, axis=0)


def kernel(query, keys, values):
    assert np.asarray(query).shape == (B, H, W, C)
    nc = _get_nc()
    in_maps = prep_in_maps(query, keys, values)
    res = run_bass_kernel_spmd(nc, in_maps, core_ids=list(range(N_CORES)))
    return assemble_out(res)
